# revision 9
# baseline (speedup 1.0000x reference)
"""Trainium2 Bass kernel for CommunityHOP GNN message passing.

Math: for each GCN branch, reference computes relu(A_hat @ (x @ W) + b) * m.
We use associativity: A_hat @ (x @ W) = (A_hat @ x) @ W, so we aggregate raw
x once per edge set (gather rows of x by edge source, one-hot matmul into the
destination block), then apply the small 256x256 weight per 128-node block.
Branch softmax-attention masks are folded into the weights/biases on the host
(relu commutes with multiplication by a positive scalar).

Sharding: nodes (and their incoming edges) are sharded by destination across
8 cores; x is replicated. The only collective is one AllGather of the first
GCN layer's output e (needed as gather source of the second GCN layer).

Perf structure (v2):
  - all gather/matmul data is bf16 (x, gathered rows, one-hot S, weights,
    branch outputs); PSUM accumulation stays f32.
  - gathers use prepare_only SWDGE descriptor generation + trigger_dma on
    4 queues, so GPSIMD only spends desc-gen time and the DMA transfers
    overlap with compute (the v1 kernel serialized the full transfer on
    GPSIMD: 99.5%% busy).
  - the one-hot S matrices are precomputed on the host for the hop edge
    sets and streamed over plain DMA; for set 0 (used twice: x-pass and
    e-pass) S is built on the DVE from (off, v) columns, balancing DMA
    bytes vs DVE time.
"""

import itertools
import math
import os
from dataclasses import dataclass, field

import numpy as np

try:
    from ml_dtypes import bfloat16 as _bf16np
except Exception:                                    # pragma: no cover
    import jax.numpy as _jnp
    _bf16np = _jnp.bfloat16


# ---------------------------------------------------------------- config

@dataclass
class Cfg:
    n: int = 50000           # nodes
    f: int = 256             # feature dim (must be 256: 2 partition tiles)
    o: int = 40              # output classes
    hops: int = 3
    ncores: int = 8
    split: int = 32768       # int16 index split for dma_gather
    s_dve_sets: tuple = (0,)  # sets whose one-hot S is built on DVE
    gather_sub: int = 8      # chunks per dma_gather call (1024 descriptors)
    scratch: int = 65536     # dynamic DMA scratch bytes/partition; the SWDGE
                             # descriptor ring holds scratch/16 descriptors, so
                             # 4 gather calls can be in flight (the v1 kernel's
                             # 16KB ring fit exactly one, serializing GPSIMD
                             # behind every transfer)

    @property
    def npc(self):
        return self.n // self.ncores

    @property
    def nblk(self):
        return (self.npc + 127) // 128


# ------------------------------------------------------------- host prep

def _prep_edges(cfg, edge_sets):
    """edge_sets: list of (src, dst) int64 (self loops NOT yet included).
    Returns (sched, idx_arr, voff_arr, s_arrs) where
      sched[k][b] = (c_lo, c_hi, idx_col, voff_col, s_col)  -- unified
      idx_arr  [ncores, 128, idxcols] int16
      voff_arr [ncores, 128, vcols] float32 (DVE sets: CB v-cols, CB off-cols)
      s_arrs   list of [128, scols] bf16 per core (DMA sets: dense one-hot S)
    """
    N, NC, NPC, NBLK, SPLIT = cfg.n, cfg.ncores, cfg.npc, cfg.nblk, cfg.split
    K = len(edge_sets)
    loops = np.arange(N, dtype=np.int64)

    per = {}   # (k,c) -> (blk, half, idx16, off, v) sorted by (blk, half)
    cnt = np.zeros((K, NC, NBLK, 2), np.int64)
    for k, (src, dst) in enumerate(edge_sets):
        src = np.concatenate([src, loops])
        dst = np.concatenate([dst, loops])
        deg = np.bincount(dst, minlength=N).astype(np.float32)
        dinv = (1.0 / np.sqrt(deg)).astype(np.float32)
        v = dinv[src] * dinv[dst]
        core = dst // NPC
        for c in range(NC):
            m = core == c
            s_c = src[m]
            d_c = dst[m] - c * NPC
            v_c = v[m]
            blk = d_c >> 7
            off = (d_c & 127).astype(np.float32)
            half = (s_c >= SPLIT).astype(np.int64)
            idx16 = (s_c - SPLIT * half).astype(np.int16)
            order = np.lexsort((half, blk))
            blk, half, idx16, off, v_c = (
                blk[order], half[order], idx16[order], off[order], v_c[order])
            per[(k, c)] = (blk, half, idx16, off, v_c)
            cnt[k, c] += np.bincount(
                blk * 2 + half, minlength=NBLK * 2).reshape(NBLK, 2)

    # unified chunk counts (max over cores), rounded up to chunks of 128
    cmax = -(-cnt.max(axis=1) // 128)          # [K, NBLK, 2]
    cb = cmax.sum(axis=2)                      # [K, NBLK]
    idx_col = np.zeros((K, NBLK), np.int64)
    voff_col = np.full((K, NBLK), -1, np.int64)
    s_col = np.full((K, NBLK), -1, np.int64)
    run = runv = runs = 0
    for k in range(K):
        dve = k in cfg.s_dve_sets
        for b in range(NBLK):
            idx_col[k, b] = run
            run += 8 * cb[k, b]
            if dve:
                voff_col[k, b] = runv
                runv += 2 * cb[k, b]
            else:
                s_col[k, b] = runs
                runs += 128 * cb[k, b]
    idxcols, vcols, scols = int(run), max(int(runv), 2), max(int(runs), 128)

    idx_arr = np.zeros((NC, 128, idxcols), np.int16)
    voff_arr = np.zeros((NC, 128, vcols), np.float32)
    s_arrs = [np.zeros((128, scols), _bf16np) for _ in range(NC)]
    for k in range(K):
        dve = k in cfg.s_dve_sets
        for c in range(NC):
            blk, half, idx16, off, v_c = per[(k, c)]
            key = blk * 2 + half
            starts = np.searchsorted(key, np.arange(NBLK * 2))
            ends = np.searchsorted(key, np.arange(NBLK * 2) + 1)
            for b in range(NBLK):
                ic0 = idx_col[k, b]
                CB = cb[k, b]
                ch0 = 0
                for h in (0, 1):
                    C = int(cmax[k, b, h])
                    if C == 0:
                        continue
                    s0, e0 = starts[b * 2 + h], ends[b * 2 + h]
                    ne = e0 - s0
                    L = C * 128
                    pidx = np.zeros(L, np.int16)
                    pidx[:ne] = idx16[s0:e0]
                    # idx: wrapped in 16 partitions, replicated to 128
                    w = pidx.reshape(L // 16, 16).T           # [16, L/16]
                    idx_arr[c, :, ic0 + 8 * ch0: ic0 + 8 * ch0 + L // 16] = (
                        np.tile(w, (8, 1)))
                    if dve:
                        vc0 = voff_col[k, b]
                        poff = np.zeros(L, np.float32)
                        poff[:ne] = off[s0:e0]
                        pv = np.zeros(L, np.float32)
                        pv[:ne] = v_c[s0:e0]
                        voff_arr[c, :, vc0 + ch0: vc0 + ch0 + C] = (
                            pv.reshape(C, 128).T)
                        voff_arr[c, :, vc0 + CB + ch0: vc0 + CB + ch0 + C] = (
                            poff.reshape(C, 128).T)
                    else:
                        # dense one-hot: S[e, (ch0+chunk)*128 + off] = v
                        sc0 = s_col[k, b]
                        r = np.arange(ne, dtype=np.int64)
                        parts = r % 128
                        cols = sc0 + (ch0 + r // 128) * 128 + (
                            off[s0:e0].astype(np.int64))
                        s_arrs[c][parts, cols] = v_c[s0:e0]
                    ch0 += C

    sched = [[(int(cmax[k, b, 0]), int(cmax[k, b, 1]), int(idx_col[k, b]),
               int(voff_col[k, b]), int(s_col[k, b]))
              for b in range(NBLK)] for k in range(K)]
    return sched, idx_arr, voff_arr, s_arrs


def _prep_all(cfg, inputs):
    """Full host-side prep. Returns (sched, in_maps)."""
    N, F, O, H = cfg.n, cfg.f, cfg.o, cfg.hops
    x = np.asarray(inputs["x"], np.float32)
    ei = np.asarray(inputs["edge_index"], np.int64)
    nei = np.asarray(inputs["new_edge_indexs"], np.int64)
    att = np.asarray(inputs["att"], np.float32)

    m = np.exp(att - att.max())
    m = (m / m.sum()).astype(np.float32)

    W_mlp = np.asarray(inputs["W_mlp"], np.float32) * m[0]
    b_mlp = np.asarray(inputs["b_mlp"], np.float32) * m[0]
    We1 = np.asarray(inputs["We1"], np.float32)
    be1 = np.asarray(inputs["be1"], np.float32)
    We2 = np.asarray(inputs["We2"], np.float32) * m[1]
    be2 = np.asarray(inputs["be2"], np.float32) * m[1]
    Wh = np.asarray(inputs["Wh"], np.float32).copy()
    bh = np.asarray(inputs["bh"], np.float32).copy()
    for i in range(H):
        Wh[i] *= m[i + 1]
        bh[i] *= m[i + 1]
    Wc = np.asarray(inputs["Wc"], np.float32)
    bc = np.asarray(inputs["bc"], np.float32)

    edge_sets = [(ei[0], ei[1])] + [(nei[i, 0], nei[i, 1]) for i in range(H)]
    sched, idx_arr, voff_arr, s_arrs = _prep_edges(cfg, edge_sets)

    x16 = np.ascontiguousarray(x.astype(_bf16np))

    # weights, branch order for lhsT form: [e2, h0, h1, h2, mlp]
    def lhsT_tiles(W):  # [2,2,128,128]: [kt][mt] = W[128kt:.., 128mt:..]
        return W.reshape(2, 128, 2, 128).transpose(0, 2, 1, 3)

    w_lhsT = np.stack([lhsT_tiles(We2)] + [lhsT_tiles(Wh[i]) for i in range(H)]
                      + [lhsT_tiles(W_mlp)]).astype(_bf16np)  # [5,2,2,128,128]
    w_rhs_e1 = We1.reshape(2, 128, F).astype(_bf16np)
    be1_row = be1.reshape(1, F).astype(_bf16np)
    ntile = (F * (H + 2)) // 128   # 10
    wcb = np.zeros((ntile + 1, 128, O), np.float32)
    wcb[:ntile] = Wc.reshape(ntile, 128, O)
    wcb[ntile, 0, :] = bc
    wcb = wcb.astype(_bf16np)
    bias_sb = np.zeros((128, 10), np.float32)   # [p, 2*branch+half]
    for bi, bv in enumerate([be2, bh[0], bh[1], bh[2], b_mlp]):
        for h in (0, 1):
            bias_sb[:, 2 * bi + h] = bv[128 * h: 128 * h + 128]
    consts = np.zeros((2, 128, 128), np.float32)
    consts[0] = np.tile(np.arange(128, dtype=np.float32)[None, :], (128, 1))
    consts[1] = np.eye(128, dtype=np.float32)

    in_maps = []
    for c in range(cfg.ncores):
        xt_own = np.ascontiguousarray(
            x[c * cfg.npc:(c + 1) * cfg.npc].T.reshape(
                2, 128, cfg.npc).astype(_bf16np))
        in_maps.append({
            "x": x16,
            "xt_own": xt_own,
            "idx": np.ascontiguousarray(idx_arr[c]),
            "voff": np.ascontiguousarray(voff_arr[c]),
            "s_arr": s_arrs[c],
            "w_lhsT": w_lhsT,
            "w_rhs_e1": w_rhs_e1,
            "be1_row": be1_row,
            "wcb": wcb,
            "bias_sb": bias_sb,
            "consts": consts,
        })
    return sched, in_maps


# --------------------------------------------------------- program build

def build_program(cfg, sched):
    import concourse.bass as bass
    import concourse.mybir as mybir
    import concourse.tile as tile
    from concourse import bacc, library_config
    from concourse.replica_groups import maybe_share_collective_output_space

    dt = mybir.dt
    f32 = dt.float32
    bf16 = dt.bfloat16
    alu = mybir.AluOpType
    act_f = mybir.ActivationFunctionType

    N, F, O, H, NC = cfg.n, cfg.f, cfg.o, cfg.hops, cfg.ncores
    NPC, NBLK, SPLIT = cfg.npc, cfg.nblk, cfg.split
    K = 1 + H
    idxcols = max(s[2] + 8 * (s[0] + s[1]) for ks in sched for s in ks)
    vcols = max([s[3] + 2 * (s[0] + s[1]) for ks in sched for s in ks
                 if s[3] >= 0] + [2])
    scols = max([s[4] + 128 * (s[0] + s[1]) for ks in sched for s in ks
                 if s[4] >= 0] + [128])
    gmax = max(max(s[0], s[1]) for ks in sched for s in ks)
    cbmax = max(s[0] + s[1] for ks in sched for s in ks)
    assert cfg.gather_sub * 128 <= (cfg.scratch // 16), cfg

    nc = bacc.Bacc("TRN2", target_bir_lowering=False, debug=False,
                   num_devices=NC, dynamic_dma_scratch_size=cfg.scratch)

    x_d = nc.dram_tensor("x", [N, F], bf16, kind="ExternalInput").ap()
    xt_d = nc.dram_tensor("xt_own", [2, 128, NPC], bf16,
                          kind="ExternalInput").ap()
    idx_d = nc.dram_tensor("idx", [128, idxcols], dt.int16,
                           kind="ExternalInput").ap()
    voff_d = nc.dram_tensor("voff", [128, vcols], f32,
                            kind="ExternalInput").ap()
    s_d = nc.dram_tensor("s_arr", [128, scols], bf16,
                         kind="ExternalInput").ap()
    wl_d = nc.dram_tensor("w_lhsT", [5, 2, 2, 128, 128], bf16,
                          kind="ExternalInput").ap()
    wr_d = nc.dram_tensor("w_rhs_e1", [2, 128, F], bf16,
                          kind="ExternalInput").ap()
    be1_d = nc.dram_tensor("be1_row", [1, F], bf16,
                           kind="ExternalInput").ap()
    wcb_d = nc.dram_tensor("wcb", [11, 128, O], bf16,
                           kind="ExternalInput").ap()
    bias_d = nc.dram_tensor("bias_sb", [128, 10], f32,
                            kind="ExternalInput").ap()
    const_d = nc.dram_tensor("consts", [2, 128, 128], f32,
                             kind="ExternalInput").ap()
    out_d = nc.dram_tensor("out_z", [NPC, O], f32, kind="ExternalOutput").ap()

    groups = [list(range(NC))]
    ag_space = maybe_share_collective_output_space("AllGather", groups)

    with tile.TileContext(nc) as tc:
        import contextlib
        ctx = contextlib.ExitStack()
        with ctx:
            const = ctx.enter_context(tc.tile_pool(name="const", bufs=1))
            g_pool = ctx.enter_context(tc.tile_pool(name="gpool", bufs=4))
            io_pool = ctx.enter_context(tc.tile_pool(name="iopool", bufs=3))
            s_pool = ctx.enter_context(tc.tile_pool(name="spool", bufs=4))
            sv_pool = ctx.enter_context(tc.tile_pool(name="svpool", bufs=6))
            sb_pool = ctx.enter_context(tc.tile_pool(name="sbpool", bufs=3))
            zcls_pool = ctx.enter_context(tc.tile_pool(name="zcls", bufs=2))
            p_agg = ctx.enter_context(
                tc.tile_pool(name="pagg", bufs=2, space="PSUM"))
            p_t = ctx.enter_context(
                tc.tile_pool(name="pt", bufs=2, space="PSUM"))
            p_w = ctx.enter_context(
                tc.tile_pool(name="pw", bufs=2, space="PSUM"))
            p_c = ctx.enter_context(
                tc.tile_pool(name="pc", bufs=2, space="PSUM"))
            dram = ctx.enter_context(
                tc.tile_pool(name="dram", bufs=1, space="DRAM"))

            # ---- constants
            nc.gpsimd.load_library(library_config.mlp)
            iota_f = const.tile([128, 128], f32)
            nc.sync.dma_start(out=iota_f[:], in_=const_d[0])
            ident_f = const.tile([128, 128], f32)
            nc.sync.dma_start(out=ident_f[:], in_=const_d[1])
            ident = const.tile([128, 128], bf16)
            nc.vector.tensor_copy(ident[:], ident_f[:])
            ones1p = const.tile([1, 128], bf16)
            nc.vector.memset(ones1p[:], 1.0)
            ones_row = const.tile([128, 128], bf16)
            nc.vector.memset(ones_row[:], 0.0)
            nc.vector.memset(ones_row[0:1, :], 1.0)

            wl = const.tile([128, 20, 128], bf16)
            for g in range(20):
                b_, kt, mt = g // 4, (g // 2) % 2, g % 2
                nc.sync.dma_start(out=wl[:, g, :], in_=wl_d[b_, kt, mt])
            wr = const.tile([128, 2, F], bf16)
            for kt in range(2):
                nc.sync.dma_start(out=wr[:, kt, :], in_=wr_d[kt])
            be1_sb = const.tile([1, F], bf16)
            nc.sync.dma_start(out=be1_sb[:], in_=be1_d[:])
            wcb = const.tile([128, 11, O], bf16)
            for t in range(11):
                nc.sync.dma_start(out=wcb[:, t, :], in_=wcb_d[t])
            bias_sb = const.tile([128, 10], f32)
            nc.sync.dma_start(out=bias_sb[:], in_=bias_d[:])

            e_loc = dram.tile([NPC, F], bf16)
            e_full = dram.tile([N, F], bf16, addr_space=ag_space)
            zt_dram = dram.tile([NBLK, 8, 128, 128], bf16)

            def gather_half(g_t, gcol, src_ap, idx_t, icol, crun):
                sub = cfg.gather_sub
                done = 0
                while done < crun:
                    take = min(sub, crun - done)
                    nc.gpsimd.dma_gather(
                        g_t[:, gcol + done:gcol + done + take, :], src_ap,
                        idx_t[:, icol + 8 * done:icol + 8 * (done + take)],
                        num_idxs=take * 128, num_idxs_reg=take * 128,
                        elem_size=F, elem_step=F)
                    done += take

            def aggregate(k, b, lo_ap, hi_ap):
                """Gather + one-hot matmul for (set k, block b).
                Returns aggT sbuf tile [128, 2, 128] bf16 (features on
                partitions)."""
                c_lo, c_hi, ic0, vc0, sc0 = sched[k][b]
                CB = c_lo + c_hi
                assert CB > 0
                dve = vc0 >= 0
                idx_t = io_pool.tile([128, 8 * cbmax], dt.int16, tag="idx")
                nc.sync.dma_start(out=idx_t[:, :8 * CB],
                                  in_=idx_d[:, ic0:ic0 + 8 * CB])
                g_t = g_pool.tile([128, cbmax, F], bf16, tag="G")
                if c_lo:
                    gather_half(g_t, 0, lo_ap, idx_t, 0, c_lo)
                if c_hi:
                    gather_half(g_t, c_lo, hi_ap, idx_t, 8 * c_lo, c_hi)
                if dve:
                    voff_t = io_pool.tile([128, 2 * cbmax], f32, tag="voff")
                    nc.sync.dma_start(out=voff_t[:, :2 * CB],
                                      in_=voff_d[:, vc0:vc0 + 2 * CB])
                else:
                    s_t = s_pool.tile([128, cbmax * 128], bf16, tag="S")
                    nc.sync.dma_start(out=s_t[:, :CB * 128],
                                      in_=s_d[:, sc0:sc0 + CB * 128])
                agg_ps = p_agg.tile([128, F], f32, tag="agg")
                for ch in range(CB):
                    if dve:
                        sc_t = sv_pool.tile([128, 128], bf16, tag="Sv")
                        nc.vector.tensor_scalar(
                            out=sc_t[:], in0=iota_f[:],
                            scalar1=voff_t[:, CB + ch:CB + ch + 1],
                            scalar2=voff_t[:, ch:ch + 1],
                            op0=alu.is_equal, op1=alu.mult)
                        lhsT = sc_t[:]
                    else:
                        lhsT = s_t[:, 128 * ch:128 * (ch + 1)]
                    nc.tensor.matmul(
                        agg_ps[:], lhsT=lhsT, rhs=g_t[:, ch, :],
                        start=(ch == 0), stop=(ch == CB - 1))
                # evacuate + transpose (bf16)
                agg_sb = sb_pool.tile([128, F], bf16, tag="aggsb")
                nc.scalar.copy(agg_sb[:], agg_ps[:])
                aggT = sb_pool.tile([128, 2, 128], bf16, tag="aggT")
                for h in (0, 1):
                    pt_ps = p_t.tile([128, 128], bf16, tag="pt")
                    nc.tensor.transpose(pt_ps[:],
                                        agg_sb[:, 128 * h:128 * (h + 1)],
                                        ident[:])
                    nc.vector.tensor_copy(aggT[:, h, :], pt_ps[:])
                return aggT

            def branch_T(rhsT, bidx, out_tiles):
                """Transposed branch: out[m] = relu(W.T @ rhs + b), m=0,1.
                rhsT: [128, 2, 128] bf16 tile; bidx into [e2,h0,h1,h2,mlp]."""
                for mt in (0, 1):
                    pw_ps = p_w.tile([128, F], f32, tag="pw")
                    nc.tensor.matmul(pw_ps[:, :128],
                                     lhsT=wl[:, (2 * bidx + 0) * 2 + mt, :],
                                     rhs=rhsT[:, 0, :], start=True, stop=False)
                    nc.tensor.matmul(pw_ps[:, :128],
                                     lhsT=wl[:, (2 * bidx + 1) * 2 + mt, :],
                                     rhs=rhsT[:, 1, :], start=False, stop=True)
                    nc.scalar.activation(
                        out_tiles[mt][:], pw_ps[:, :128], act_f.Relu,
                        bias=bias_sb[:, 2 * bidx + mt:2 * bidx + mt + 1],
                        scale=1.0)

            def block_rows(b):
                nr = min(128, NPC - b * 128)
                return b * 128, nr

            x_lo = x_d[0:SPLIT, :]
            x_hi = x_d[SPLIT:N, :]

            # ---------------- phase 1a: set 0 (ei) -> e rows + allgather
            for b in range(NBLK):
                r0, nr = block_rows(b)
                aggT = aggregate(0, b, x_lo, x_hi)
                pe_ps = p_w.tile([128, F], f32, tag="pw")
                nc.tensor.matmul(pe_ps[:], lhsT=aggT[:, 0, :], rhs=wr[:, 0, :],
                                 start=True, stop=False)
                nc.tensor.matmul(pe_ps[:], lhsT=aggT[:, 1, :], rhs=wr[:, 1, :],
                                 start=False, stop=False)
                nc.tensor.matmul(pe_ps[:], lhsT=ones1p[:, :], rhs=be1_sb[:, :],
                                 start=False, stop=True)
                e_sb = sb_pool.tile([128, F], bf16, tag="esb")
                nc.scalar.activation(e_sb[:], pe_ps[:], act_f.Relu)
                nc.sync.dma_start(out=e_loc[r0:r0 + nr, :], in_=e_sb[:nr, :])

            nc.gpsimd.collective_compute(
                "AllGather", alu.bypass, replica_groups=groups,
                ins=[e_loc[:].opt()], outs=[e_full[:].opt()])

            # ---------------- phase 1b: hop sets 1..3 (+ mlp on last set)
            for k in range(1, K):
                for b in range(NBLK):
                    aggT = aggregate(k, b, x_lo, x_hi)
                    zt0 = sb_pool.tile([128, 128], bf16, tag="zt", bufs=6)
                    zt1 = sb_pool.tile([128, 128], bf16, tag="zt", bufs=6)
                    branch_T(aggT, k, [zt0, zt1])  # bidx: h0=1,h1=2,h2=3
                    slot = 2 * (k - 1)
                    nc.scalar.dma_start(out=zt_dram[b, slot], in_=zt0[:])
                    nc.scalar.dma_start(out=zt_dram[b, slot + 1], in_=zt1[:])
                    if k == K - 1:
                        r0, nr = block_rows(b)
                        xtt = sb_pool.tile([128, 2, 128], bf16, tag="xtt")
                        if nr < 128:
                            nc.vector.memset(xtt[:], 0.0)
                        for kt in (0, 1):
                            nc.sync.dma_start(out=xtt[:, kt, :nr],
                                              in_=xt_d[kt, :, r0:r0 + nr])
                        zm0 = sb_pool.tile([128, 128], bf16, tag="zt", bufs=6)
                        zm1 = sb_pool.tile([128, 128], bf16, tag="zt", bufs=6)
                        branch_T(xtt, 4, [zm0, zm1])
                        nc.scalar.dma_start(out=zt_dram[b, 6], in_=zm0[:])
                        nc.scalar.dma_start(out=zt_dram[b, 7], in_=zm1[:])

            # ---------------- phase 2: e2 branch + classifier + log_softmax
            e_lo = e_full[0:SPLIT, :]
            e_hi = e_full[SPLIT:N, :]
            for b in range(NBLK):
                r0, nr = block_rows(b)
                aggT2 = aggregate(0, b, e_lo, e_hi)
                e2t0 = zcls_pool.tile([128, 128], bf16, tag="zcls", bufs=12)
                e2t1 = zcls_pool.tile([128, 128], bf16, tag="zcls", bufs=12)
                branch_T(aggT2, 0, [e2t0, e2t1])
                zts = []
                for t in range(8):
                    zz = zcls_pool.tile([128, 128], bf16, tag="zcls", bufs=12)
                    nc.sync.dma_start(out=zz[:], in_=zt_dram[b, t])
                    zts.append(zz)
                # z tile order: h0(0,1) h1(2,3) h2(4,5) e2(6,7) mlp(8,9)
                order = [zts[0], zts[1], zts[2], zts[3], zts[4], zts[5],
                         e2t0, e2t1, zts[6], zts[7]]
                pc_ps = p_c.tile([128, O], f32, tag="pcls")
                for t in range(10):
                    nc.tensor.matmul(pc_ps[:], lhsT=order[t][:],
                                     rhs=wcb[:, t, :],
                                     start=(t == 0), stop=False)
                nc.tensor.matmul(pc_ps[:], lhsT=ones_row[:], rhs=wcb[:, 10, :],
                                 start=False, stop=True)
                mx = sv_pool.tile([128, 1], f32, tag="mx")
                nc.vector.tensor_reduce(mx[:], pc_ps[:],
                                        axis=mybir.AxisListType.X, op=alu.max)
                tt = sv_pool.tile([128, O], f32, tag="tt")
                nc.vector.tensor_scalar(out=tt[:], in0=pc_ps[:],
                                        scalar1=mx[:, 0:1], scalar2=None,
                                        op0=alu.subtract)
                ex = sv_pool.tile([128, O], f32, tag="ex")
                se = sv_pool.tile([128, 1], f32, tag="se")
                nc.scalar.activation(ex[:], tt[:], act_f.Exp,
                                     accum_out=se[:])
                lse = sv_pool.tile([128, 1], f32, tag="lse")
                nc.scalar.activation(lse[:], se[:], act_f.Ln)
                ot = sv_pool.tile([128, O], f32, tag="ot")
                nc.vector.tensor_scalar(out=ot[:], in0=tt[:],
                                        scalar1=lse[:, 0:1], scalar2=None,
                                        op0=alu.subtract)
                nc.sync.dma_start(out=out_d[r0:r0 + nr, :], in_=ot[:nr, :])

    nc.compile()
    return nc


# ------------------------------------------------------------------ main

def _run(cfg, inputs, trace=False):
    from concourse.bass_utils import run_bass_kernel_spmd

    sched, in_maps = _prep_all(cfg, inputs)
    nc = build_program(cfg, sched)
    res = run_bass_kernel_spmd(nc, in_maps, list(range(cfg.ncores)),
                               trace=trace)
    out = np.concatenate([res.results[c]["out_z"]
                          for c in range(cfg.ncores)], axis=0)
    return out, res


def kernel(**inputs) -> np.ndarray:
    cfg = Cfg()
    out, _ = _run(cfg, inputs, trace=False)
    return out
